# revision 9
# baseline (speedup 1.0000x reference)
"""Windowed multi-head attention (Swin-style) for 8 Trainium2 NeuronCores.

Problem: x [1024, 64, 512], mask [256, 64, 64], H=16 heads, D=32.
Data-parallel over windows: core c gets windows [128c, 128c+128) and mask
shard mask[(c%2)*128 : (c%2)*128+128] (window b uses mask[b % 256]).

This environment rejects matmuls with nonzero tile_position columns and
hangs on transpose-mode matmuls, so the kernel uses only full-column
matmuls (out base partition 0) and host-side transposes:

Host: xT [B,C,N]; expmaskT2 [B/2, 128, 128] f16 = exp(maskT) pair-stacked
block-diagonal (off-diagonal blocks = 0) so cross-window score garbage is
killed exactly; wq/wkv/wp natural; bias reshapes.

Per-core dataflow (128 windows, quads of 4, window-pairs inside):
  XT [128(c), 4kc, 256(4w,64t)] <-- DMA
  QT = Wq^T X^T (fp32r, N=256) -> f16 [128(4h,32d), mc, 256]; KT likewise
  V2 = X Wv (lhsT=XT pair cols) [128(2w,64t), 512] -> f16
  ST[h] = KT_h^T QT_h per pair: lhsT [32, 128(2w,64m)], rhs [32, 128(2w,64n)]
     -> st [128(2w,64m), 4hq, 128(2w,64n)] (cross-window blocks = garbage)
  E = exp(st) (ACT->f16); A = E * expmaskT2 (DVE, bcast over heads;
     garbage blocks -> exact 0, so A is block-diagonal)
  sigA[h] = ones^T A_h (M=128 broadcast rows) -> [128, 4hq, 128]
  An = A * recip(sigA) (DVE recip + mul)
  OT_h = (V2_h)^T An_h: lhsT=v2[:,32h:+32] [128(m2w),32], rhs=An_h
     -> ot [32(d), 4hq, 128(2w,64n)] per mc; PSUM->SBUF copy (fp32r)
  assemble otn2 [128(4h,32d), kc, 128] via 16 partition-shift SBUF DMAs
  Y = OTn^T Wp + bp (4 fp32r MMs K=128 + ones-row bias) -> y
"""
import sys

sys.path.insert(0, "/opt/trn_rl_repo")

import numpy as np

N = 64
C = 512
H = 16
D = 32
KC = 4
SCALE = D ** -0.5
NW = 128        # windows per core
N_CORES = 8
JUNK = -1e4     # off-diagonal mask fill; exp -> exact 0


def build_attention(tc, y, xt, emask2, wq, bq2, wkv, bkvk2, bkvv, wp, bp1, nw,
                    big_fp32r=True, small_dt="f16", shift_qk=True):
    """Emit the kernel into TileContext tc.

    DRAM APs: y [nw,64,512] out; xt [nw,512,64]; emask2 [nw/2,128,128] f16
    (exp of pair-stacked block-diag maskT); wq/wkv/wp natural;
    bq2/bkvk2 [128,KC] chunk-major biases; bkvv/bp1 [1,512] bias rows.
    shift_qk: copy QT/KT head-rows to partition-0 tiles via SBUF DMA
    (needed when nonzero tile_position rows are rejected by the runtime).
    """
    import concourse.bass as bass
    from concourse import mybir
    from contextlib import ExitStack

    FP32 = mybir.dt.float32
    FP32R = mybir.dt.float32r

    nc = tc.nc
    assert nw % 4 == 0
    nq = nw // 4
    a_dt = {"f16": mybir.dt.float16, "bf16": mybir.dt.bfloat16}[small_dt]
    bd = FP32R if big_fp32r else FP32

    def bsrc(ap):
        return ap.bitcast(FP32R) if big_fp32r else ap

    ctx = ExitStack()
    with ctx:
        consts = ctx.enter_context(tc.tile_pool(name="consts", bufs=1))
        sbuf = ctx.enter_context(tc.tile_pool(name="sbuf", bufs=1))
        ring = ctx.enter_context(tc.tile_pool(name="ring", bufs=7, space="PSUM"))

        # ---- constants -------------------------------------------------
        wq_sb = consts.tile([128, KC, C], bd)
        nc.sync.dma_start(wq_sb, bsrc(wq.rearrange("(kc p) c -> p kc c", p=128)))
        wkv_sb = consts.tile([128, KC, 2 * C], bd)
        nc.sync.dma_start(wkv_sb, bsrc(wkv.rearrange("(kc p) c -> p kc c", p=128)))
        wp_sb = consts.tile([128, KC, C], bd)
        nc.sync.dma_start(wp_sb, bsrc(wp.rearrange("(kc p) c -> p kc c", p=128)))

        bqs_sb = consts.tile([128, KC], FP32)
        nc.sync.dma_start(bqs_sb, bq2[:, 0:KC])
        nc.scalar.mul(bqs_sb, bqs_sb, SCALE)
        bkvk_sb = consts.tile([128, KC], FP32)
        nc.sync.dma_start(bkvk_sb, bkvk2[:, 0:KC])
        bkvv_row = consts.tile([1, C], bd)
        nc.sync.dma_start(bkvv_row, bsrc(bkvv[0:1, :]))
        bp_row = consts.tile([1, C], bd)
        nc.sync.dma_start(bp_row, bsrc(bp1[0:1, :]))

        ones_f = consts.tile([128, 128], FP32)
        nc.vector.memset(ones_f, 1.0)
        ones_r = consts.tile([1, 128], bd)
        nc.scalar.copy(ones_r, ones_f[0:1, 0:128])
        ones_a = consts.tile([128, 128], a_dt)
        nc.vector.memset(ones_a, 1.0)

        # exp-mask pair tiles [128(2w,64m), mp, 128(2w,64n)] f16
        nwp = nw // 2
        emask_sb = consts.tile([128, nwp, 128], a_dt)
        nc.sync.dma_start(emask_sb, emask2.rearrange("mp m n -> m mp n"))

        # ---- main loop over quads (4 windows) -------------------------
        for q in range(nq):
            w0 = 4 * q
            xt_sb = sbuf.tile([128, KC, 4, N], bd, tag="xt", bufs=3,
                              name=f"xt_sb_{q}")
            for kc in range(KC):
                nc.sync.dma_start(
                    xt_sb[:, kc, :, :],
                    bsrc(xt[w0:w0 + 4, 128 * kc:128 * kc + 128, :]
                         .rearrange("w p t -> p w t")))
            xtf = xt_sb.rearrange("p kc w t -> p kc (w t)")

            # QT/KT [128(4h,32d), mc, 256(4w,64t)] f16
            qt_sb = sbuf.tile([128, KC, 256], a_dt, tag="qt", bufs=2,
                              name=f"qt_sb_{q}")
            kt_sb = sbuf.tile([128, KC, 256], a_dt, tag="kt", bufs=2,
                              name=f"kt_sb_{q}")
            for mc in range(KC):
                qt_ps = ring.tile([128, 256], FP32, tag="ring", name=f"qt_ps_{q}_{mc}")
                for kc in range(KC):
                    nc.tensor.matmul(qt_ps,
                                     wq_sb[:, kc, 128 * mc:128 * mc + 128],
                                     xtf[:, kc, :],
                                     start=(kc == 0), stop=(kc == KC - 1))
                nc.scalar.activation(qt_sb[:, mc, :], qt_ps,
                                     mybir.ActivationFunctionType.Identity,
                                     bias=bqs_sb[:, mc:mc + 1], scale=SCALE)
                kt_ps = ring.tile([128, 256], FP32, tag="ring", name=f"kt_ps_{q}_{mc}")
                for kc in range(KC):
                    nc.tensor.matmul(kt_ps,
                                     wkv_sb[:, kc, 128 * mc:128 * mc + 128],
                                     xtf[:, kc, :],
                                     start=(kc == 0), stop=(kc == KC - 1))
                nc.scalar.activation(kt_sb[:, mc, :], kt_ps,
                                     mybir.ActivationFunctionType.Identity,
                                     bias=bkvk_sb[:, mc:mc + 1], scale=1.0)

            if shift_qk:
                # head-rows to partition 0: qt2/kt2 [32, 4j, KC*256]
                qt2 = sbuf.tile([32, 4, KC, 256], a_dt, tag="qt2", bufs=2,
                                name=f"qt2_{q}")
                kt2 = sbuf.tile([32, 4, KC, 256], a_dt, tag="kt2", bufs=2,
                                name=f"kt2_{q}")
                for j in range(4):
                    nc.sync.dma_start(qt2[:, j, :, :], qt_sb[32 * j:32 * j + 32, :, :])
                    nc.sync.dma_start(kt2[:, j, :, :], kt_sb[32 * j:32 * j + 32, :, :])

                def qk(h, mc, sl):
                    return (qt2[:, h % 4, mc, sl], kt2[:, h % 4, mc, sl])
            else:
                def qk(h, mc, sl):
                    j = h % 4
                    return (qt_sb[32 * j:32 * j + 32, mc, sl],
                            kt_sb[32 * j:32 * j + 32, mc, sl])

            for p in range(2):  # window pairs
                mp = 2 * q + p
                psl = slice(128 * p, 128 * p + 128)  # pair token cols in quad

                # V2 [128(2w,64t), 512c] f16
                v2_ps = ring.tile([128, C], FP32, tag="ring", name=f"v2_ps_{q}_{p}")
                for kc in range(KC):
                    nc.tensor.matmul(v2_ps, xtf[:, kc, psl],
                                     wkv_sb[:, kc, C:2 * C],
                                     start=(kc == 0), stop=False)
                nc.tensor.matmul(v2_ps, ones_r[0:1, 0:128], bkvv_row,
                                 start=False, stop=True)
                v2_sb = sbuf.tile([128, C], a_dt, tag="v2", bufs=3,
                                  name=f"v2_sb_{q}_{p}")
                nc.vector.tensor_copy(v2_sb, v2_ps)

                a_ts, sg_ts = [], []
                for t in range(4):  # 4 heads per tile
                    st_ps = ring.tile([128, 4, 128], FP32, tag="ring",
                                      name=f"st_ps_{q}_{p}_{t}")
                    for hq in range(4):
                        h = 4 * t + hq
                        qsl, ksl = qk(h, h // 4, psl)
                        nc.tensor.matmul(st_ps[:, hq, :], ksl, qsl,
                                         tile_position=(0, 0) if shift_qk
                                         else (32 * (h % 4), 0))
                    # E = exp(st) -> f16; A = E * expmask (kills garbage)
                    a_sb = sbuf.tile([128, 4, 128], a_dt, tag="a", bufs=8,
                                     name=f"a_sb_{q}_{p}_{t}")
                    nc.scalar.activation(a_sb, st_ps,
                                         mybir.ActivationFunctionType.Exp)
                    em = emask_sb[:, mp, :]
                    em_bc = bass.AP(tensor=em.tensor, offset=em.offset,
                                    ap=[em.ap[0], [0, 4], em.ap[-1]])
                    nc.vector.tensor_mul(a_sb, a_sb, em_bc)
                    # sigA[h]: ones^T A_h, broadcast over all 128 rows
                    sg_ps = ring.tile([128, 4, 128], FP32, tag="ring",
                                      name=f"sg_ps_{q}_{p}_{t}")
                    for hq in range(4):
                        nc.tensor.matmul(sg_ps[:, hq, :], ones_a,
                                         a_sb[:, hq, :], tile_position=(0, 0))
                    rec_sb = sbuf.tile([128, 4, 128], FP32, tag="rec", bufs=4,
                                       name=f"rec_sb_{q}_{p}_{t}")
                    nc.vector.reciprocal(rec_sb, sg_ps)
                    nc.vector.tensor_mul(a_sb, a_sb, rec_sb)
                    a_ts.append(a_sb)

                # OT chunk-packed in PSUM via column positions:
                # otc [128(4hq,32d), mc, 128(2w,64n)]
                otc_ps = ring.tile([128, KC, 128], FP32, tag="ring",
                                   name=f"otc_ps_{q}_{p}")
                for mc in range(KC):
                    for hq in range(4):
                        h = 4 * mc + hq
                        nc.tensor.matmul(
                            otc_ps[32 * hq:32 * hq + 32, mc, :],
                            v2_sb[:, 32 * h:32 * h + 32],
                            a_ts[h // 4][:, h % 4, :], tile_position=(0, 32 * hq))
                otn2 = sbuf.tile([128, KC, 128], bd, tag="otn2", bufs=2,
                                 name=f"otn2_{q}_{p}")
                nc.vector.tensor_copy(otn2[:, 0:2, :], otc_ps[:, 0:2, :])
                nc.scalar.copy(otn2[:, 2:4, :], otc_ps[:, 2:4, :])

                # proj: Y [128(2w,64t), 512]
                y_ps = ring.tile([128, C], FP32, tag="ring", name=f"y_ps_{q}_{p}")
                for kc in range(KC):
                    nc.tensor.matmul(y_ps, otn2[:, kc, :], wp_sb[:, kc, :],
                                     start=(kc == 0), stop=False)
                nc.tensor.matmul(y_ps, ones_r[0:1, 0:128], bp_row,
                                 start=False, stop=True)
                y_sb = sbuf.tile([128, C], FP32, tag="y", bufs=3,
                                 name=f"y_sb_{q}_{p}")
                nc.scalar.copy(y_sb, y_ps)
                nc.sync.dma_start(
                    y[w0 + 2 * p:w0 + 2 * p + 2].flatten_outer_dims(), y_sb)


_CACHE = {}


def _build_module(nw=NW, **flags):
    key = (nw, tuple(sorted(flags.items())))
    if key in _CACHE:
        return _CACHE[key]
    import concourse.tile as tile
    from concourse import bacc, mybir

    FP32 = mybir.dt.float32
    F16 = mybir.dt.float16
    nc = bacc.Bacc("TRN2", target_bir_lowering=False, debug=False)
    d = {}
    shapes = {
        "xt": ([nw, C, N], FP32), "emask2": ([nw // 2, 128, 128], F16),
        "wq": ([C, C], FP32), "bq2": ([128, KC], FP32),
        "wkv": ([C, 2 * C], FP32), "bkvk2": ([128, KC], FP32),
        "bkvv": ([1, C], FP32), "wp": ([C, C], FP32), "bp1": ([1, C], FP32),
    }
    for name, (shape, dt) in shapes.items():
        d[name] = nc.dram_tensor(name, shape, dt, kind="ExternalInput")
    d_y = nc.dram_tensor("y", [nw, N, C], FP32, kind="ExternalOutput")

    with tile.TileContext(nc) as tc:
        build_attention(tc, d_y[:], d["xt"][:], d["emask2"][:], d["wq"][:],
                        d["bq2"][:], d["wkv"][:], d["bkvk2"][:], d["bkvv"][:],
                        d["wp"][:], d["bp1"][:], nw, **flags)
    nc.compile()
    _CACHE[key] = nc
    return nc


def make_in_maps(inputs, nw=NW, n_cores=N_CORES):
    """Host-side preprocessing + per-core sharding."""
    import ml_dtypes
    x = np.asarray(inputs["x"], dtype=np.float32)
    mask = np.asarray(inputs["mask"], dtype=np.float32)
    xt = np.ascontiguousarray(x.transpose(0, 2, 1))          # [B, C, N]
    maskt = mask.transpose(0, 2, 1)                          # [nW, m, n]
    nmask = maskt.shape[0]
    # pair-stacked block-diagonal exp-mask [nW/2, 128, 128] f16
    em = np.zeros((nmask // 2, 128, 128), dtype=np.float32)
    em[:, 0:64, 0:64] = maskt[0::2]
    em[:, 64:128, 64:128] = maskt[1::2]
    em[:, 0:64, 64:128] = JUNK
    em[:, 64:128, 0:64] = JUNK
    em2 = np.exp(em).astype(np.float16)
    bq = np.asarray(inputs["bq"], dtype=np.float32)
    bkv = np.asarray(inputs["bkv"], dtype=np.float32)
    bp = np.asarray(inputs["bp"], dtype=np.float32)
    wq = np.ascontiguousarray(np.asarray(inputs["Wq"], dtype=np.float32))
    wkv = np.ascontiguousarray(np.asarray(inputs["Wkv"], dtype=np.float32))
    wp = np.ascontiguousarray(np.asarray(inputs["Wp"], dtype=np.float32))
    bq2 = np.ascontiguousarray(bq.reshape(KC, 128).T)
    bkvk2 = np.ascontiguousarray(bkv[:C].reshape(KC, 128).T)
    bkvv = np.ascontiguousarray(bkv[C:].reshape(1, C))
    bp1 = np.ascontiguousarray(bp.reshape(1, C))
    in_maps = []
    for c in range(n_cores):
        m0 = ((c * nw) % nmask) // 2
        in_maps.append({
            "xt": np.ascontiguousarray(xt[c * nw:(c + 1) * nw]),
            "emask2": np.ascontiguousarray(em2[m0:m0 + nw // 2]),
            "wq": wq, "bq2": bq2, "wkv": wkv, "bkvk2": bkvk2,
            "bkvv": bkvv, "wp": wp, "bp1": bp1,
        })
    return in_maps


def kernel(**inputs):
    from concourse.bass_utils import run_bass_kernel_spmd

    nc = _build_module()
    in_maps = make_in_maps(inputs)
    res = run_bass_kernel_spmd(nc, in_maps, core_ids=list(range(N_CORES)))
    return np.concatenate([r["y"] for r in res.results], axis=0)


# revision 10
# speedup vs baseline: 1.4636x; 1.4636x over previous
"""Windowed multi-head attention (Swin-style) for 8 Trainium2 NeuronCores.

Problem: x [1024, 64, 512], mask [256, 64, 64], H=16 heads, D=32.
Data-parallel over windows: core c gets windows [128c, 128c+128) and mask
shard mask[(c%2)*128 : (c%2)*128+128] (window b uses mask[b % 256]).

This environment rejects matmuls with nonzero tile_position columns and
hangs on transpose-mode matmuls, so the kernel uses only full-column
matmuls (out base partition 0) and host-side transposes:

Host: xT [B,C,N]; expmaskT2 [B/2, 128, 128] f16 = exp(maskT) pair-stacked
block-diagonal (off-diagonal blocks = 0) so cross-window score garbage is
killed exactly; wq/wkv/wp natural; bias reshapes.

Per-core dataflow (128 windows, quads of 4, window-pairs inside):
  XT [128(c), 4kc, 256(4w,64t)] <-- DMA
  QT = Wq^T X^T (fp32r, N=256) -> f16 [128(4h,32d), mc, 256]; KT likewise
  V2 = X Wv (lhsT=XT pair cols) [128(2w,64t), 512] -> f16
  ST[h] = KT_h^T QT_h per pair: lhsT [32, 128(2w,64m)], rhs [32, 128(2w,64n)]
     -> st [128(2w,64m), 4hq, 128(2w,64n)] (cross-window blocks = garbage)
  E = exp(st) (ACT->f16); A = E * expmaskT2 (DVE, bcast over heads;
     garbage blocks -> exact 0, so A is block-diagonal)
  sigA[h] = ones^T A_h (M=128 broadcast rows) -> [128, 4hq, 128]
  An = A * recip(sigA) (DVE recip + mul)
  OT_h = (V2_h)^T An_h: lhsT=v2[:,32h:+32] [128(m2w),32], rhs=An_h
     -> ot [32(d), 4hq, 128(2w,64n)] per mc; PSUM->SBUF copy (fp32r)
  assemble otn2 [128(4h,32d), kc, 128] via 16 partition-shift SBUF DMAs
  Y = OTn^T Wp + bp (4 fp32r MMs K=128 + ones-row bias) -> y
"""
import sys

sys.path.insert(0, "/opt/trn_rl_repo")

import numpy as np

N = 64
C = 512
H = 16
D = 32
KC = 4
SCALE = D ** -0.5
NW = 128        # windows per core
N_CORES = 8
JUNK = -1e4     # off-diagonal mask fill; exp -> exact 0


def build_attention(tc, y, xt, emask2, wq, bq2, wkv, bkvk2, bkvv, wp, bp1, nw,
                    big_fp32r=True, small_dt="f16", shift_qk=True):
    """Emit the kernel into TileContext tc.

    DRAM APs: y [nw,64,512] out; xt [nw,512,64]; emask2 [nw/2,128,128] f16
    (exp of pair-stacked block-diag maskT); wq/wkv/wp natural;
    bq2/bkvk2 [128,KC] chunk-major biases; bkvv/bp1 [1,512] bias rows.
    shift_qk: copy QT/KT head-rows to partition-0 tiles via SBUF DMA
    (needed when nonzero tile_position rows are rejected by the runtime).
    """
    import concourse.bass as bass
    from concourse import mybir
    from contextlib import ExitStack

    FP32 = mybir.dt.float32
    FP32R = mybir.dt.float32r

    nc = tc.nc
    assert nw % 4 == 0
    nq = nw // 4
    a_dt = {"f16": mybir.dt.float16, "bf16": mybir.dt.bfloat16}[small_dt]
    bd = FP32R if big_fp32r else FP32

    def bsrc(ap):
        return ap.bitcast(FP32R) if big_fp32r else ap

    ctx = ExitStack()
    with ctx:
        consts = ctx.enter_context(tc.tile_pool(name="consts", bufs=1))
        sbuf = ctx.enter_context(tc.tile_pool(name="sbuf", bufs=1))
        ring = ctx.enter_context(tc.tile_pool(name="ring", bufs=7, space="PSUM"))

        # ---- constants -------------------------------------------------
        wq_sb = consts.tile([128, KC, C], bd)
        nc.sync.dma_start(wq_sb, bsrc(wq.rearrange("(kc p) c -> p kc c", p=128)))
        wkv_sb = consts.tile([128, KC, 2 * C], bd)
        nc.sync.dma_start(wkv_sb, bsrc(wkv.rearrange("(kc p) c -> p kc c", p=128)))
        wp_sb = consts.tile([128, KC, C], bd)
        nc.sync.dma_start(wp_sb, bsrc(wp.rearrange("(kc p) c -> p kc c", p=128)))

        bqs_sb = consts.tile([128, KC], FP32)
        nc.sync.dma_start(bqs_sb, bq2[:, 0:KC])
        nc.scalar.mul(bqs_sb, bqs_sb, SCALE)
        bkvk_sb = consts.tile([128, KC], FP32)
        nc.sync.dma_start(bkvk_sb, bkvk2[:, 0:KC])
        bkvv_row = consts.tile([1, C], bd)
        nc.sync.dma_start(bkvv_row, bsrc(bkvv[0:1, :]))
        bp_row = consts.tile([1, C], bd)
        nc.sync.dma_start(bp_row, bsrc(bp1[0:1, :]))

        ones_f = consts.tile([128, 128], FP32)
        nc.vector.memset(ones_f, 1.0)
        ones_r = consts.tile([1, 128], bd)
        nc.scalar.copy(ones_r, ones_f[0:1, 0:128])
        ones_a = consts.tile([128, 128], a_dt)
        nc.vector.memset(ones_a, 1.0)

        # exp-mask pair tiles [128(2w,64m), mp, 128(2w,64n)] f16
        nwp = nw // 2
        emask_sb = consts.tile([128, nwp, 128], a_dt)
        nc.sync.dma_start(emask_sb, emask2.rearrange("mp m n -> m mp n"))

        # ---- main loop over quads (4 windows) -------------------------
        for q in range(nq):
            w0 = 4 * q
            xt_sb = sbuf.tile([128, KC, 4, N], bd, tag="xt", bufs=3,
                              name=f"xt_sb_{q}")
            for kc in range(KC):
                nc.sync.dma_start(
                    xt_sb[:, kc, :, :],
                    bsrc(xt[w0:w0 + 4, 128 * kc:128 * kc + 128, :]
                         .rearrange("w p t -> p w t")))
            xtf = xt_sb.rearrange("p kc w t -> p kc (w t)")

            # QT/KT [128(4h,32d), mc, 256(4w,64t)] f16
            qt_sb = sbuf.tile([128, KC, 256], a_dt, tag="qt", bufs=2,
                              name=f"qt_sb_{q}")
            kt_sb = sbuf.tile([128, KC, 256], a_dt, tag="kt", bufs=2,
                              name=f"kt_sb_{q}")
            for mc in range(KC):
                qt_ps = ring.tile([128, 256], FP32, tag="ring", name=f"qt_ps_{q}_{mc}")
                for kc in range(KC):
                    nc.tensor.matmul(qt_ps,
                                     wq_sb[:, kc, 128 * mc:128 * mc + 128],
                                     xtf[:, kc, :],
                                     start=(kc == 0), stop=(kc == KC - 1))
                nc.scalar.activation(qt_sb[:, mc, :], qt_ps,
                                     mybir.ActivationFunctionType.Identity,
                                     bias=bqs_sb[:, mc:mc + 1], scale=SCALE)
                kt_ps = ring.tile([128, 256], FP32, tag="ring", name=f"kt_ps_{q}_{mc}")
                for kc in range(KC):
                    nc.tensor.matmul(kt_ps,
                                     wkv_sb[:, kc, 128 * mc:128 * mc + 128],
                                     xtf[:, kc, :],
                                     start=(kc == 0), stop=(kc == KC - 1))
                nc.scalar.activation(kt_sb[:, mc, :], kt_ps,
                                     mybir.ActivationFunctionType.Identity,
                                     bias=bkvk_sb[:, mc:mc + 1], scale=1.0)

            if shift_qk:
                # head-rows to partition 0: qt2/kt2 [32, 4j, KC*256]
                qt2 = sbuf.tile([32, 4, KC, 256], a_dt, tag="qt2", bufs=2,
                                name=f"qt2_{q}")
                kt2 = sbuf.tile([32, 4, KC, 256], a_dt, tag="kt2", bufs=2,
                                name=f"kt2_{q}")
                for j in range(4):
                    nc.sync.dma_start(qt2[:, j, :, :], qt_sb[32 * j:32 * j + 32, :, :])
                    nc.sync.dma_start(kt2[:, j, :, :], kt_sb[32 * j:32 * j + 32, :, :])

                def qk(h, mc, sl):
                    return (qt2[:, h % 4, mc, sl], kt2[:, h % 4, mc, sl])
            else:
                def qk(h, mc, sl):
                    j = h % 4
                    return (qt_sb[32 * j:32 * j + 32, mc, sl],
                            kt_sb[32 * j:32 * j + 32, mc, sl])

            for p in range(2):  # window pairs
                mp = 2 * q + p
                psl = slice(128 * p, 128 * p + 128)  # pair token cols in quad

                # V2 [128(2w,64t), 512c] f16
                v2_ps = ring.tile([128, C], FP32, tag="ring", name=f"v2_ps_{q}_{p}")
                for kc in range(KC):
                    nc.tensor.matmul(v2_ps, xtf[:, kc, psl],
                                     wkv_sb[:, kc, C:2 * C],
                                     start=(kc == 0), stop=False)
                nc.tensor.matmul(v2_ps, ones_r[0:1, 0:128], bkvv_row,
                                 start=False, stop=True)
                v2_sb = sbuf.tile([128, C], a_dt, tag="v2", bufs=3,
                                  name=f"v2_sb_{q}_{p}")
                nc.vector.tensor_copy(v2_sb, v2_ps)

                a_ts, sg_ts = [], []
                for t in range(4):  # 4 heads per tile
                    st_ps = ring.tile([128, 4, 128], FP32, tag="ring",
                                      name=f"st_ps_{q}_{p}_{t}")
                    for hq in range(4):
                        h = 4 * t + hq
                        qsl, ksl = qk(h, h // 4, psl)
                        nc.tensor.matmul(st_ps[:, hq, :], ksl, qsl,
                                         tile_position=(0, 0) if shift_qk
                                         else (32 * (h % 4), 0))
                    # E = exp(st) -> f16; A = E * expmask (kills garbage)
                    a_sb = sbuf.tile([128, 4, 128], a_dt, tag="a", bufs=8,
                                     name=f"a_sb_{q}_{p}_{t}")
                    nc.scalar.activation(a_sb, st_ps,
                                         mybir.ActivationFunctionType.Exp)
                    em = emask_sb[:, mp, :]
                    em_bc = bass.AP(tensor=em.tensor, offset=em.offset,
                                    ap=[em.ap[0], [0, 4], em.ap[-1]])
                    nc.vector.tensor_mul(a_sb, a_sb, em_bc)
                    # sigA: ones^T A for all 4 head slots in one matmul
                    sg_ps = ring.tile([128, 4, 128], FP32, tag="ring",
                                      name=f"sg_ps_{q}_{p}_{t}")
                    nc.tensor.matmul(sg_ps.rearrange("p a b -> p (a b)"), ones_a,
                                     a_sb.rearrange("p a b -> p (a b)"),
                                     tile_position=(0, 0))
                    rec_sb = sbuf.tile([128, 4, 128], FP32, tag="rec", bufs=4,
                                       name=f"rec_sb_{q}_{p}_{t}")
                    nc.vector.reciprocal_approx_fast(rec_sb, sg_ps)
                    nc.vector.tensor_mul(a_sb, a_sb, rec_sb)
                    a_ts.append(a_sb)

                # OT chunk-packed in PSUM via column positions:
                # otc [128(4hq,32d), mc, 128(2w,64n)]
                otc_ps = ring.tile([128, KC, 128], FP32, tag="ring",
                                   name=f"otc_ps_{q}_{p}")
                for mc in range(KC):
                    for hq in range(4):
                        h = 4 * mc + hq
                        nc.tensor.matmul(
                            otc_ps[32 * hq:32 * hq + 32, mc, :],
                            v2_sb[:, 32 * h:32 * h + 32],
                            a_ts[h // 4][:, h % 4, :], tile_position=(0, 32 * hq))
                otn2 = sbuf.tile([128, KC, 128], bd, tag="otn2", bufs=2,
                                 name=f"otn2_{q}_{p}")
                nc.vector.tensor_copy(otn2[:, 0:2, :], otc_ps[:, 0:2, :])
                nc.scalar.copy(otn2[:, 2:4, :], otc_ps[:, 2:4, :])

                # proj: Y [128(2w,64t), 512]
                y_ps = ring.tile([128, C], FP32, tag="ring", name=f"y_ps_{q}_{p}")
                for kc in range(KC):
                    nc.tensor.matmul(y_ps, otn2[:, kc, :], wp_sb[:, kc, :],
                                     start=(kc == 0), stop=False)
                nc.tensor.matmul(y_ps, ones_r[0:1, 0:128], bp_row,
                                 start=False, stop=True)
                y_sb = sbuf.tile([128, C], FP32, tag="y", bufs=3,
                                 name=f"y_sb_{q}_{p}")
                nc.scalar.copy(y_sb, y_ps)
                nc.sync.dma_start(
                    y[w0 + 2 * p:w0 + 2 * p + 2].flatten_outer_dims(), y_sb)


_CACHE = {}


def _build_module(nw=NW, **flags):
    key = (nw, tuple(sorted(flags.items())))
    if key in _CACHE:
        return _CACHE[key]
    import concourse.tile as tile
    from concourse import bacc, mybir

    FP32 = mybir.dt.float32
    F16 = mybir.dt.float16
    nc = bacc.Bacc("TRN2", target_bir_lowering=False, debug=False)
    d = {}
    shapes = {
        "xt": ([nw, C, N], FP32), "emask2": ([nw // 2, 128, 128], F16),
        "wq": ([C, C], FP32), "bq2": ([128, KC], FP32),
        "wkv": ([C, 2 * C], FP32), "bkvk2": ([128, KC], FP32),
        "bkvv": ([1, C], FP32), "wp": ([C, C], FP32), "bp1": ([1, C], FP32),
    }
    for name, (shape, dt) in shapes.items():
        d[name] = nc.dram_tensor(name, shape, dt, kind="ExternalInput")
    d_y = nc.dram_tensor("y", [nw, N, C], FP32, kind="ExternalOutput")

    with tile.TileContext(nc) as tc:
        build_attention(tc, d_y[:], d["xt"][:], d["emask2"][:], d["wq"][:],
                        d["bq2"][:], d["wkv"][:], d["bkvk2"][:], d["bkvv"][:],
                        d["wp"][:], d["bp1"][:], nw, **flags)
    nc.compile()
    _CACHE[key] = nc
    return nc


def make_in_maps(inputs, nw=NW, n_cores=N_CORES):
    """Host-side preprocessing + per-core sharding."""
    import ml_dtypes
    x = np.asarray(inputs["x"], dtype=np.float32)
    mask = np.asarray(inputs["mask"], dtype=np.float32)
    xt = np.ascontiguousarray(x.transpose(0, 2, 1))          # [B, C, N]
    maskt = mask.transpose(0, 2, 1)                          # [nW, m, n]
    nmask = maskt.shape[0]
    # pair-stacked block-diagonal exp-mask [nW/2, 128, 128] f16
    em = np.zeros((nmask // 2, 128, 128), dtype=np.float32)
    em[:, 0:64, 0:64] = maskt[0::2]
    em[:, 64:128, 64:128] = maskt[1::2]
    em[:, 0:64, 64:128] = JUNK
    em[:, 64:128, 0:64] = JUNK
    em2 = np.exp(em).astype(np.float16)
    bq = np.asarray(inputs["bq"], dtype=np.float32)
    bkv = np.asarray(inputs["bkv"], dtype=np.float32)
    bp = np.asarray(inputs["bp"], dtype=np.float32)
    wq = np.ascontiguousarray(np.asarray(inputs["Wq"], dtype=np.float32))
    wkv = np.ascontiguousarray(np.asarray(inputs["Wkv"], dtype=np.float32))
    wp = np.ascontiguousarray(np.asarray(inputs["Wp"], dtype=np.float32))
    bq2 = np.ascontiguousarray(bq.reshape(KC, 128).T)
    bkvk2 = np.ascontiguousarray(bkv[:C].reshape(KC, 128).T)
    bkvv = np.ascontiguousarray(bkv[C:].reshape(1, C))
    bp1 = np.ascontiguousarray(bp.reshape(1, C))
    in_maps = []
    for c in range(n_cores):
        m0 = ((c * nw) % nmask) // 2
        in_maps.append({
            "xt": np.ascontiguousarray(xt[c * nw:(c + 1) * nw]),
            "emask2": np.ascontiguousarray(em2[m0:m0 + nw // 2]),
            "wq": wq, "bq2": bq2, "wkv": wkv, "bkvk2": bkvk2,
            "bkvv": bkvv, "wp": wp, "bp1": bp1,
        })
    return in_maps


def kernel(**inputs):
    from concourse.bass_utils import run_bass_kernel_spmd

    nc = _build_module()
    in_maps = make_in_maps(inputs)
    res = run_bass_kernel_spmd(nc, in_maps, core_ids=list(range(N_CORES)))
    return np.concatenate([r["y"] for r in res.results], axis=0)


# revision 11
# speedup vs baseline: 2.6703x; 1.8245x over previous
"""Windowed multi-head attention (Swin-style) for 8 Trainium2 NeuronCores.

Problem: x [1024, 64, 512], mask [256, 64, 64], H=16 heads, D=32.
Data-parallel over windows: core c gets windows [128c, 128c+128) and mask
shard mask[(c%2)*128 : (c%2)*128+128] (window b uses mask[b % 256]).

This environment rejects matmuls with nonzero tile_position columns and
hangs on transpose-mode matmuls, so the kernel uses only full-column
matmuls (out base partition 0) and host-side transposes:

Host: xT [B,C,N]; expmaskT2 [B/2, 128, 128] f16 = exp(maskT) pair-stacked
block-diagonal (off-diagonal blocks = 0) so cross-window score garbage is
killed exactly; wq/wkv/wp natural; bias reshapes.

Per-core dataflow (128 windows, quads of 4, window-pairs inside):
  XT [128(c), 4kc, 256(4w,64t)] <-- DMA
  QT = Wq^T X^T (fp32r, N=256) -> f16 [128(4h,32d), mc, 256]; KT likewise
  V2 = X Wv (lhsT=XT pair cols) [128(2w,64t), 512] -> f16
  ST[h] = KT_h^T QT_h per pair: lhsT [32, 128(2w,64m)], rhs [32, 128(2w,64n)]
     -> st [128(2w,64m), 4hq, 128(2w,64n)] (cross-window blocks = garbage)
  E = exp(st) (ACT->f16); A = E * expmaskT2 (DVE, bcast over heads;
     garbage blocks -> exact 0, so A is block-diagonal)
  sigA[h] = ones^T A_h (M=128 broadcast rows) -> [128, 4hq, 128]
  An = A * recip(sigA) (DVE recip + mul)
  OT_h = (V2_h)^T An_h: lhsT=v2[:,32h:+32] [128(m2w),32], rhs=An_h
     -> ot [32(d), 4hq, 128(2w,64n)] per mc; PSUM->SBUF copy (fp32r)
  assemble otn2 [128(4h,32d), kc, 128] via 16 partition-shift SBUF DMAs
  Y = OTn^T Wp + bp (4 fp32r MMs K=128 + ones-row bias) -> y
"""
import sys

sys.path.insert(0, "/opt/trn_rl_repo")

import numpy as np

N = 64
C = 512
H = 16
D = 32
KC = 4
SCALE = D ** -0.5
NW = 128        # windows per core
N_CORES = 8
JUNK = -1e4     # off-diagonal mask fill; exp -> exact 0


def build_attention(tc, y, xt, emask2, wq, bq2, wkv, bkvk2, bkvv, wp, bp1, nw,
                    big_fp32r=True, small_dt="f16", shift_qk=True):
    """Emit the kernel into TileContext tc.

    DRAM APs: y [nw,64,512] out; xt [nw,512,64]; emask2 [nw/2,128,128] f16
    (exp of pair-stacked block-diag maskT); wq/wkv/wp natural;
    bq2/bkvk2 [128,KC] chunk-major biases; bkvv/bp1 [1,512] bias rows.
    shift_qk: copy QT/KT head-rows to partition-0 tiles via SBUF DMA
    (needed when nonzero tile_position rows are rejected by the runtime).
    """
    import concourse.bass as bass
    from concourse import mybir
    from contextlib import ExitStack

    FP32 = mybir.dt.float32
    FP32R = mybir.dt.float32r

    nc = tc.nc
    assert nw % 4 == 0
    nq = nw // 4
    a_dt = {"f16": mybir.dt.float16, "bf16": mybir.dt.bfloat16}[small_dt]
    bd = FP32R if big_fp32r else FP32

    def bsrc(ap):
        return ap.bitcast(FP32R) if big_fp32r else ap

    ctx = ExitStack()
    with ctx:
        consts = ctx.enter_context(tc.tile_pool(name="consts", bufs=1))
        sbuf = ctx.enter_context(tc.tile_pool(name="sbuf", bufs=1))
        ring = ctx.enter_context(tc.tile_pool(name="ring", bufs=2, space="PSUM"))

        # ---- constants -------------------------------------------------
        wq_sb = consts.tile([128, KC, C], bd)
        nc.sync.dma_start(wq_sb, bsrc(wq.rearrange("(kc p) c -> p kc c", p=128)))
        wkv_sb = consts.tile([128, KC, 2 * C], bd)
        nc.sync.dma_start(wkv_sb, bsrc(wkv.rearrange("(kc p) c -> p kc c", p=128)))
        wp_sb = consts.tile([128, KC, C], bd)
        nc.sync.dma_start(wp_sb, bsrc(wp.rearrange("(kc p) c -> p kc c", p=128)))

        bqs_sb = consts.tile([128, KC], FP32)
        nc.sync.dma_start(bqs_sb, bq2[:, 0:KC])
        nc.scalar.mul(bqs_sb, bqs_sb, SCALE)
        bkvk_sb = consts.tile([128, KC], FP32)
        nc.sync.dma_start(bkvk_sb, bkvk2[:, 0:KC])
        bkvv_row = consts.tile([1, C], bd)
        nc.sync.dma_start(bkvv_row, bsrc(bkvv[0:1, :]))
        bp_row = consts.tile([1, C], bd)
        nc.sync.dma_start(bp_row, bsrc(bp1[0:1, :]))

        ones_f = consts.tile([128, 128], FP32)
        nc.vector.memset(ones_f, 1.0)
        ones_r = consts.tile([1, 128], bd)
        nc.scalar.copy(ones_r, ones_f[0:1, 0:128])
        ones_a = consts.tile([128, 128], a_dt)
        nc.vector.memset(ones_a, 1.0)

        # exp-mask pair tiles [128(2w,64m), mp, 128(2w,64n)] f16
        nwp = nw // 2
        emask_sb = consts.tile([128, nwp, 128], a_dt)
        nc.sync.dma_start(emask_sb, emask2.rearrange("mp m n -> m mp n"))

        # ---- main loop over quads (4 windows) -------------------------
        for q in range(nq):
            w0 = 4 * q
            xt_sb = sbuf.tile([128, KC, 4, N], bd, tag="xt", bufs=3,
                              name=f"xt_sb_{q}")
            for kc in range(KC):
                nc.sync.dma_start(
                    xt_sb[:, kc, :, :],
                    bsrc(xt[w0:w0 + 4, 128 * kc:128 * kc + 128, :]
                         .rearrange("w p t -> p w t")))
            xtf = xt_sb.rearrange("p kc w t -> p kc (w t)")

            # QT/KT [128(4h,32d), mc, 256(4w,64t)] f16
            qt_sb = sbuf.tile([128, KC, 256], a_dt, tag="qt", bufs=2,
                              name=f"qt_sb_{q}")
            kt_sb = sbuf.tile([128, KC, 256], a_dt, tag="kt", bufs=2,
                              name=f"kt_sb_{q}")
            for mc in range(KC):
                qt_ps = ring.tile([128, 256], FP32, tag="ps_qk", name=f"qt_ps_{q}_{mc}")
                for kc in range(KC):
                    nc.tensor.matmul(qt_ps,
                                     wq_sb[:, kc, 128 * mc:128 * mc + 128],
                                     xtf[:, kc, :],
                                     start=(kc == 0), stop=(kc == KC - 1))
                nc.scalar.activation(qt_sb[:, mc, :], qt_ps,
                                     mybir.ActivationFunctionType.Identity,
                                     bias=bqs_sb[:, mc:mc + 1], scale=SCALE)
                kt_ps = ring.tile([128, 256], FP32, tag="ps_qk", name=f"kt_ps_{q}_{mc}")
                for kc in range(KC):
                    nc.tensor.matmul(kt_ps,
                                     wkv_sb[:, kc, 128 * mc:128 * mc + 128],
                                     xtf[:, kc, :],
                                     start=(kc == 0), stop=(kc == KC - 1))
                nc.scalar.activation(kt_sb[:, mc, :], kt_ps,
                                     mybir.ActivationFunctionType.Identity,
                                     bias=bkvk_sb[:, mc:mc + 1], scale=1.0)

            if shift_qk:
                # head-rows to partition 0: qt2/kt2 [32, 4j, KC*256]
                qt2 = sbuf.tile([32, 4, KC, 256], a_dt, tag="qt2", bufs=2,
                                name=f"qt2_{q}")
                kt2 = sbuf.tile([32, 4, KC, 256], a_dt, tag="kt2", bufs=2,
                                name=f"kt2_{q}")
                for j in range(4):
                    nc.sync.dma_start(qt2[:, j, :, :], qt_sb[32 * j:32 * j + 32, :, :])
                    nc.sync.dma_start(kt2[:, j, :, :], kt_sb[32 * j:32 * j + 32, :, :])

                def qk(h, mc, sl):
                    return (qt2[:, h % 4, mc, sl], kt2[:, h % 4, mc, sl])
            else:
                def qk(h, mc, sl):
                    j = h % 4
                    return (qt_sb[32 * j:32 * j + 32, mc, sl],
                            kt_sb[32 * j:32 * j + 32, mc, sl])

            for p in range(2):  # window pairs
                mp = 2 * q + p
                psl = slice(128 * p, 128 * p + 128)  # pair token cols in quad

                # V2 [128(2w,64t), 512c] f16
                v2_ps = ring.tile([128, C], FP32, tag="ps_io", name=f"v2_ps_{q}_{p}")
                for kc in range(KC):
                    nc.tensor.matmul(v2_ps, xtf[:, kc, psl],
                                     wkv_sb[:, kc, C:2 * C],
                                     start=(kc == 0), stop=False)
                nc.tensor.matmul(v2_ps, ones_r[0:1, 0:128], bkvv_row,
                                 start=False, stop=True)
                v2_sb = sbuf.tile([128, C], a_dt, tag="v2", bufs=3,
                                  name=f"v2_sb_{q}_{p}")
                nc.vector.tensor_copy(v2_sb, v2_ps)

                a_ts, sg_ts = [], []
                for t in range(4):  # 4 heads per tile
                    st_ps = ring.tile([128, 4, 128], FP32, tag="ps_st",
                                      name=f"st_ps_{q}_{p}_{t}")
                    for hq in range(4):
                        h = 4 * t + hq
                        qsl, ksl = qk(h, h // 4, psl)
                        nc.tensor.matmul(st_ps[:, hq, :], ksl, qsl,
                                         tile_position=(0, 0) if shift_qk
                                         else (32 * (h % 4), 0))
                    # E = exp(st) -> f16; A = E * expmask (kills garbage)
                    a_sb = sbuf.tile([128, 4, 128], a_dt, tag="a", bufs=8,
                                     name=f"a_sb_{q}_{p}_{t}")
                    nc.scalar.activation(a_sb, st_ps,
                                         mybir.ActivationFunctionType.Exp)
                    em = emask_sb[:, mp, :]
                    em_bc = bass.AP(tensor=em.tensor, offset=em.offset,
                                    ap=[em.ap[0], [0, 4], em.ap[-1]])
                    nc.vector.tensor_mul(a_sb, a_sb, em_bc)
                    # sigA: ones^T A for all 4 head slots in one matmul
                    sg_ps = ring.tile([128, 4, 128], FP32, tag="ps_sg",
                                      name=f"sg_ps_{q}_{p}_{t}")
                    nc.tensor.matmul(sg_ps.rearrange("p a b -> p (a b)"), ones_a,
                                     a_sb.rearrange("p a b -> p (a b)"),
                                     tile_position=(0, 0))
                    rec_sb = sbuf.tile([128, 4, 128], FP32, tag="rec", bufs=4,
                                       name=f"rec_sb_{q}_{p}_{t}")
                    nc.vector.reciprocal_approx_fast(rec_sb, sg_ps)
                    nc.vector.tensor_mul(a_sb, a_sb, rec_sb)
                    a_ts.append(a_sb)

                # OT chunk-packed in PSUM via column positions:
                # otc [128(4hq,32d), mc, 128(2w,64n)]
                otc_ps = ring.tile([128, KC, 128], FP32, tag="ps_io",
                                   name=f"otc_ps_{q}_{p}")
                for mc in range(KC):
                    for hq in range(4):
                        h = 4 * mc + hq
                        nc.tensor.matmul(
                            otc_ps[32 * hq:32 * hq + 32, mc, :],
                            v2_sb[:, 32 * h:32 * h + 32],
                            a_ts[h // 4][:, h % 4, :], tile_position=(0, 32 * hq))
                otn2 = sbuf.tile([128, KC, 128], bd, tag="otn2", bufs=2,
                                 name=f"otn2_{q}_{p}")
                nc.vector.tensor_copy(otn2[:, 0:2, :], otc_ps[:, 0:2, :])
                nc.scalar.copy(otn2[:, 2:4, :], otc_ps[:, 2:4, :])

                # proj: Y [128(2w,64t), 512]
                y_ps = ring.tile([128, C], FP32, tag="ps_io", name=f"y_ps_{q}_{p}")
                for kc in range(KC):
                    nc.tensor.matmul(y_ps, otn2[:, kc, :], wp_sb[:, kc, :],
                                     start=(kc == 0), stop=False)
                nc.tensor.matmul(y_ps, ones_r[0:1, 0:128], bp_row,
                                 start=False, stop=True)
                y_sb = sbuf.tile([128, C], FP32, tag="y", bufs=3,
                                 name=f"y_sb_{q}_{p}")
                nc.scalar.copy(y_sb, y_ps)
                nc.sync.dma_start(
                    y[w0 + 2 * p:w0 + 2 * p + 2].flatten_outer_dims(), y_sb)


_CACHE = {}


def _build_module(nw=NW, **flags):
    key = (nw, tuple(sorted(flags.items())))
    if key in _CACHE:
        return _CACHE[key]
    import concourse.tile as tile
    from concourse import bacc, mybir

    FP32 = mybir.dt.float32
    F16 = mybir.dt.float16
    nc = bacc.Bacc("TRN2", target_bir_lowering=False, debug=False)
    d = {}
    shapes = {
        "xt": ([nw, C, N], FP32), "emask2": ([nw // 2, 128, 128], F16),
        "wq": ([C, C], FP32), "bq2": ([128, KC], FP32),
        "wkv": ([C, 2 * C], FP32), "bkvk2": ([128, KC], FP32),
        "bkvv": ([1, C], FP32), "wp": ([C, C], FP32), "bp1": ([1, C], FP32),
    }
    for name, (shape, dt) in shapes.items():
        d[name] = nc.dram_tensor(name, shape, dt, kind="ExternalInput")
    d_y = nc.dram_tensor("y", [nw, N, C], FP32, kind="ExternalOutput")

    with tile.TileContext(nc) as tc:
        build_attention(tc, d_y[:], d["xt"][:], d["emask2"][:], d["wq"][:],
                        d["bq2"][:], d["wkv"][:], d["bkvk2"][:], d["bkvv"][:],
                        d["wp"][:], d["bp1"][:], nw, **flags)
    nc.compile()
    _CACHE[key] = nc
    return nc


def make_in_maps(inputs, nw=NW, n_cores=N_CORES):
    """Host-side preprocessing + per-core sharding."""
    import ml_dtypes
    x = np.asarray(inputs["x"], dtype=np.float32)
    mask = np.asarray(inputs["mask"], dtype=np.float32)
    xt = np.ascontiguousarray(x.transpose(0, 2, 1))          # [B, C, N]
    maskt = mask.transpose(0, 2, 1)                          # [nW, m, n]
    nmask = maskt.shape[0]
    # pair-stacked block-diagonal exp-mask [nW/2, 128, 128] f16
    em = np.zeros((nmask // 2, 128, 128), dtype=np.float32)
    em[:, 0:64, 0:64] = maskt[0::2]
    em[:, 64:128, 64:128] = maskt[1::2]
    em[:, 0:64, 64:128] = JUNK
    em[:, 64:128, 0:64] = JUNK
    em2 = np.exp(em).astype(np.float16)
    bq = np.asarray(inputs["bq"], dtype=np.float32)
    bkv = np.asarray(inputs["bkv"], dtype=np.float32)
    bp = np.asarray(inputs["bp"], dtype=np.float32)
    wq = np.ascontiguousarray(np.asarray(inputs["Wq"], dtype=np.float32))
    wkv = np.ascontiguousarray(np.asarray(inputs["Wkv"], dtype=np.float32))
    wp = np.ascontiguousarray(np.asarray(inputs["Wp"], dtype=np.float32))
    bq2 = np.ascontiguousarray(bq.reshape(KC, 128).T)
    bkvk2 = np.ascontiguousarray(bkv[:C].reshape(KC, 128).T)
    bkvv = np.ascontiguousarray(bkv[C:].reshape(1, C))
    bp1 = np.ascontiguousarray(bp.reshape(1, C))
    in_maps = []
    for c in range(n_cores):
        m0 = ((c * nw) % nmask) // 2
        in_maps.append({
            "xt": np.ascontiguousarray(xt[c * nw:(c + 1) * nw]),
            "emask2": np.ascontiguousarray(em2[m0:m0 + nw // 2]),
            "wq": wq, "bq2": bq2, "wkv": wkv, "bkvk2": bkvk2,
            "bkvv": bkvv, "wp": wp, "bp1": bp1,
        })
    return in_maps


def kernel(**inputs):
    from concourse.bass_utils import run_bass_kernel_spmd

    nc = _build_module()
    in_maps = make_in_maps(inputs)
    res = run_bass_kernel_spmd(nc, in_maps, core_ids=list(range(N_CORES)))
    return np.concatenate([r["y"] for r in res.results], axis=0)
